# revision 5
# baseline (speedup 1.0000x reference)
"""Multi-head attention (B=2, L=S=2048, D=1024, H=16) on 8 Trainium2 cores.

Sharding: core c -> batch b = c // 4, head group g = c % 4 (4 heads per core).
W_Q/K/V column-sharded (256 cols per core), W_O row-sharded (256 rows per core);
the 4 partial outputs per batch are summed on the host (plus bias terms).

Per-core pipeline (all big tensors kept transposed so no on-device transposes):
  projections: QT = 0.125*(x Wq + bq)^T, KT = (x Wk + bk)^T (feature-major
    [256, L]); Vaug = [V_h | ones] per head (seq-major, fp16), V bias folded
    out on the host (softmax rows sum to 1 => + bv @ Wo + bo once).
  attention, per (l-tile 512, s-tile 128): S^T = KT^T QT (row-packed pairs of
    heads, K=64, the two 64-row matmuls co-execute in disjoint PE row
    halves); E = exp(S^T) * maskT; T_h += Vaug_h^T E accumulates the head
    output AND its softmax row-sums in one matmul (ones columns act as the
    reducer).
  The AV accumulation runs TWO s-tiles behind the score matmuls (software
  pipelining) so by the time the PE reaches an AV matmul its E operand is
  already computed - the PE never idle-waits on the ACT exp / DVE mask
  chain.  Idle waits were re-throttling the PE clock to 1.2 GHz (HAM
  activity monitor) for ~36% of matmuls in the unpipelined version.
  Engine balance: ACT does exp only (QT bias moved to DVE); the 0/1 mask
  multiply runs on DVE with a subset of s-tiles offloaded to GpSimd.
  out-projection: out_partial = outT^T Wo_rows; PSUM->SBUF copies split
  between ACT (free after the last exp) and DVE.

All matmul operands fp16 (1 cyc/row); PSUM fp32.  PSUM budget 8 banks =
scores 2x2 + T_h 4x1; projection and output-projection matmuls borrow the
same slots.
"""
from contextlib import ExitStack

import numpy as np

import concourse.bass as bass
import concourse.mybir as mybir
import concourse.tile as tile
from concourse import bacc
from concourse.bass_utils import run_bass_kernel_spmd

F16 = mybir.dt.float16
F32 = mybir.dt.float32

D = 1024          # d_model
H = 16            # heads
DK = 64           # head dim
B, L = 2, 2048
NCORES = 8
HPC = 4           # heads per core
FPC = HPC * DK    # features per core = 256
KD = D // 128     # 8 contraction subtiles for projections
LT, LTW = 4, 512  # l tiles
ST, STW = 16, 128  # s tiles
Exp = mybir.ActivationFunctionType.Exp

# s-tiles whose mask multiply runs on GpSimd instead of DVE (DVE relief).
GPS_STS = frozenset({5, 11})

_CACHED_NC = None


def _build():
    nc = bacc.Bacc("TRN2", target_bir_lowering=False, debug=False,
                   num_devices=NCORES)
    xT = nc.declare_dram_parameter("xT", [128, KD, L], F16, isOutput=False)
    wq = nc.declare_dram_parameter("wq", [128, KD, FPC], F16, isOutput=False)
    wk = nc.declare_dram_parameter("wk", [128, KD, FPC], F16, isOutput=False)
    wv = nc.declare_dram_parameter("wv", [128, KD, FPC], F16, isOutput=False)
    wo = nc.declare_dram_parameter("wo", [128, 2, D], F16, isOutput=False)
    bq = nc.declare_dram_parameter("bq", [128, 2], F32, isOutput=False)
    bk = nc.declare_dram_parameter("bk", [128, 2], F32, isOutput=False)
    maskT = nc.declare_dram_parameter("maskT", [ST, LT, 128, LTW], F16,
                                      isOutput=False)
    out = nc.declare_dram_parameter("out", [128, ST, D], F16, isOutput=True)

    with tile.TileContext(nc) as tc, ExitStack() as ctx:
        pool = ctx.enter_context(tc.tile_pool(name="pers", bufs=1))
        mpool = ctx.enter_context(tc.tile_pool(name="mpool", bufs=4))
        epool = ctx.enter_context(tc.tile_pool(name="epool", bufs=4))
        rbpool = ctx.enter_context(tc.tile_pool(name="rbpool", bufs=4))
        opool = ctx.enter_context(tc.tile_pool(name="opool", bufs=3))
        scp = ctx.enter_context(tc.tile_pool(name="scp", bufs=2, space="PSUM"))
        tp = ctx.enter_context(tc.tile_pool(name="tp", bufs=1, space="PSUM"))

        xt = pool.tile([128, KD, L], F16)
        wq_sb = pool.tile([128, KD, FPC], F16)
        wk_sb = pool.tile([128, KD, FPC], F16)
        wv_sb = pool.tile([128, KD, FPC], F16)
        wo_sb = pool.tile([128, 2, D], F16)
        bq_sb = pool.tile([128, 2], F32)
        bk_sb = pool.tile([128, 2], F32)
        nc.sync.dma_start(out=wk_sb[:], in_=wk[:])
        nc.sync.dma_start(out=xt[:, 0, :], in_=xT[:, 0, :])
        nc.sync.dma_start(out=wv_sb[:], in_=wv[:])
        for kd in range(1, KD):
            nc.sync.dma_start(out=xt[:, kd, :], in_=xT[:, kd, :])
        nc.sync.dma_start(out=wq_sb[:], in_=wq[:])
        nc.sync.dma_start(out=bk_sb[:], in_=bk[:])
        nc.sync.dma_start(out=bq_sb[:], in_=bq[:])
        nc.sync.dma_start(out=wo_sb[:], in_=wo[:])

        QT = pool.tile([128, 2, L], F16)   # [feat(2x128), l]: Q^T * 0.125
        KT = pool.tile([128, 2, L], F16)
        # Vaug[:, st, h]: even h -> [V_h | 1], odd h -> [1 | V_h]
        Vaug = pool.tile([128, ST, HPC, 128], F16)
        nc.gpsimd.memset(Vaug[:], 1.0)
        outTs = [pool.tile([128, 2, LTW], F16, name=f"outT{i}")
                 for i in range(LT)]

        def emit_kt_chunk(c):
            lsl = slice(c * LTW, (c + 1) * LTW)
            ps = scp.tile([128, 2, LTW], F32, tag="sc", name=f"pk{c}")
            for ft in range(2):
                fsl = slice(ft * 128, (ft + 1) * 128)
                for kd in range(KD):
                    nc.tensor.matmul(ps[:, ft, :], wk_sb[:, kd, fsl],
                                     xt[:, kd, lsl],
                                     start=(kd == 0), stop=(kd == KD - 1))
                nc.vector.scalar_tensor_tensor(
                    KT[:, ft, lsl], ps[:, ft, :], 1.0,
                    bk_sb[:, ft:ft + 1].to_broadcast((128, LTW)),
                    mybir.AluOpType.mult, mybir.AluOpType.add)

        def emit_v_chunk(c):
            for st in range(4 * c, 4 * c + 4):
                ssl = slice(st * STW, (st + 1) * STW)
                psv = tp.tile([128, LTW], F32, tag=f"T{st % 4}", name=f"psv{st}")
                for kd in range(KD):
                    nc.tensor.matmul(psv[:, :FPC], xt[:, kd, ssl],
                                     wv_sb[:, kd, :],
                                     start=(kd == 0), stop=(kd == KD - 1))
                for h in range(HPC):
                    off = 0 if h % 2 == 0 else 64
                    nc.vector.tensor_copy(Vaug[:, st, h, off:off + 64],
                                          psv[:, DK * h:DK * (h + 1)])

        def emit_qt(lt):
            lsl = slice(lt * LTW, (lt + 1) * LTW)
            psq = scp.tile([128, 2, LTW], F32, tag="sc", name=f"pq{lt}")
            for ft in range(2):
                fsl = slice(ft * 128, (ft + 1) * 128)
                for kd in range(KD):
                    nc.tensor.matmul(psq[:, ft, :], wq_sb[:, kd, fsl],
                                     xt[:, kd, lsl],
                                     start=(kd == 0), stop=(kd == KD - 1))
                nc.vector.scalar_tensor_tensor(
                    QT[:, ft, lsl], psq[:, ft, :], 0.125,
                    bq_sb[:, ft:ft + 1].to_broadcast((128, LTW)),
                    mybir.AluOpType.mult, mybir.AluOpType.add)

        for c in range(4):
            emit_kt_chunk(c)
            emit_v_chunk(c)
        emit_qt(0)

        for lt in range(LT):
            lsl = slice(lt * LTW, (lt + 1) * LTW)
            if lt > 0:
                emit_qt(lt)
            Ts = [tp.tile([128, LTW], F32, tag=f"T{h}", name=f"T{h}_{lt}")
                  for h in range(HPC)]
            mks = {}
            Es = {}

            def issue_mk(st):
                mk = mpool.tile([128, LTW], F16, tag="mk")
                nc.sync.dma_start(out=mk[:], in_=maskT[st, lt])
                mks[st] = mk

            def emit_scores(st):
                ssl = slice(st * STW, (st + 1) * STW)
                mk = mks.pop(st)
                Epair = []
                for pair in range(2):
                    sc = scp.tile([128, 2, LTW], F32, tag="sc")
                    for i in range(2):
                        nc.tensor.matmul(
                            sc[:, i, :],
                            KT[64 * i:64 * (i + 1), pair, ssl],
                            QT[64 * i:64 * (i + 1), pair, lsl],
                            start=True, stop=True)
                    E = epool.tile([128, 2, LTW], F16, tag=f"E{pair}")
                    nc.scalar.activation(E[:], sc[:], Exp)
                    eng = nc.gpsimd if st in GPS_STS else nc.vector
                    eng.tensor_mul(
                        E[:], E[:],
                        mk[:, None, :].to_broadcast((128, 2, LTW)))
                    Epair.append(E)
                Es[st] = Epair

            def emit_av(st):
                Epair = Es.pop(st)
                for pair in range(2):
                    for i in range(2):
                        h = 2 * pair + i
                        nc.tensor.matmul(Ts[h][:], Vaug[:, st, h, :],
                                         Epair[pair][:, i, :],
                                         start=(st == 0), stop=(st == ST - 1))

            issue_mk(0)
            issue_mk(1)
            emit_scores(0)
            emit_scores(1)
            for st in range(ST):
                if st + 2 < ST:
                    issue_mk(st + 2)
                    emit_scores(st + 2)
                emit_av(st)
            for h in range(HPC):
                # reciprocal_approx_fast only works at partition base 0, so
                # route the row sums through lanes 0:64 in both parities.
                pair, i = divmod(h, 2)
                av_sl = slice(64 * i, 64 * (i + 1))        # av lanes
                rs_sl = slice(64 * (1 - i), 64 * (2 - i))  # row-sum lanes
                rb = rbpool.tile([128, LTW], F32)
                if i == 0:   # av 0:64, sums 64:128 -> move sums down first
                    nc.vector.tensor_copy(rb[64:128, :], Ts[h][rs_sl, :])
                    nc.gpsimd.dma_start(out=rb[0:64, :], in_=rb[64:128, :])
                    nc.vector.reciprocal_approx_fast(out=rb[0:64, :],
                                                     in_=rb[0:64, :])
                else:        # sums 0:64 -> recip at base 0, then move up
                    nc.vector.reciprocal_approx_fast(out=rb[0:64, :],
                                                     in_=Ts[h][rs_sl, :])
                    nc.gpsimd.dma_start(out=rb[64:128, :], in_=rb[0:64, :])
                nc.vector.tensor_mul(outTs[lt][av_sl, pair, :],
                                     Ts[h][av_sl, :], rb[av_sl, :])

        # ---------------- output projection ----------------
        for lt8 in range(ST):
            ps3 = scp.tile([128, 2, LTW], F32, tag="sc", name=f"ps3_{lt8}")
            for nf in range(2):
                nsl = slice(nf * 512, (nf + 1) * 512)
                for pair in range(2):
                    nc.tensor.matmul(
                        ps3[:, nf, :],
                        outTs[lt8 // 4][:, pair,
                                        (lt8 % 4) * 128:(lt8 % 4 + 1) * 128],
                        wo_sb[:, pair, nsl],
                        start=(pair == 0), stop=(pair == 1))
            ob = opool.tile([128, D], F16)
            if lt8 % 2 == 0:
                nc.scalar.copy(ob[:], ps3[:])
            else:
                nc.vector.tensor_copy(ob[:], ps3[:])
            nc.gpsimd.dma_start(out=out[:, lt8, :], in_=ob[:])

    nc.compile()
    return nc


def _get_nc():
    global _CACHED_NC
    if _CACHED_NC is None:
        _CACHED_NC = _build()
    return _CACHED_NC


def _prep_core_inputs(c, x, mask, Wq, bq, Wk, bk, Wv, Wo):
    b, g = divmod(c, 4)
    cs = slice(g * FPC, (g + 1) * FPC)

    xT = np.ascontiguousarray(
        x[b].T.reshape(KD, 128, L).transpose(1, 0, 2)).astype(np.float16)
    wq_c = np.ascontiguousarray(
        Wq[:, cs].reshape(KD, 128, FPC).transpose(1, 0, 2)).astype(np.float16)
    wk_c = np.ascontiguousarray(
        Wk[:, cs].reshape(KD, 128, FPC).transpose(1, 0, 2)).astype(np.float16)
    wv_c = np.ascontiguousarray(
        Wv[:, cs].reshape(KD, 128, FPC).transpose(1, 0, 2)).astype(np.float16)
    wo_c = np.ascontiguousarray(
        Wo[cs, :].reshape(2, 128, D).transpose(1, 0, 2)).astype(np.float16)
    bq_c = np.ascontiguousarray(
        (bq[cs] * 0.125).reshape(2, 128).T).astype(np.float32)
    bk_c = np.ascontiguousarray(bk[cs].reshape(2, 128).T).astype(np.float32)
    mT = mask[b].astype(np.float16).T  # [S, L]
    maskT = np.ascontiguousarray(
        mT.reshape(ST, 128, LT, LTW).transpose(0, 2, 1, 3))
    return {"xT": xT, "wq": wq_c, "wk": wk_c, "wv": wv_c, "wo": wo_c,
            "bq": bq_c, "bk": bk_c, "maskT": maskT}


def kernel(x, mask, Wq, bq, Wk, bk, Wv, bv, Wo, bo):
    x = np.asarray(x, np.float32)
    mask = np.asarray(mask)
    Wq, bq = np.asarray(Wq, np.float32), np.asarray(bq, np.float32)
    Wk, bk = np.asarray(Wk, np.float32), np.asarray(bk, np.float32)
    Wv, bv = np.asarray(Wv, np.float32), np.asarray(bv, np.float32)
    Wo, bo = np.asarray(Wo, np.float32), np.asarray(bo, np.float32)

    nc = _get_nc()
    in_maps = [_prep_core_inputs(c, x, mask, Wq, bq, Wk, bk, Wv, Wo)
               for c in range(NCORES)]
    res = run_bass_kernel_spmd(nc, in_maps, list(range(NCORES)))

    const_vec = (bv @ Wo + bo).astype(np.float32)  # A rows sum to 1
    outs = []
    for b in range(B):
        acc = np.zeros((L, D), np.float32)
        for g in range(4):
            part = res.results[4 * b + g]["out"]  # [128, 16, 1024] fp16
            acc += part.transpose(1, 0, 2).reshape(L, D).astype(np.float32)
        acc += const_vec
        outs.append(acc)
    return np.stack(outs)


# revision 6
# speedup vs baseline: 1.1864x; 1.1864x over previous
"""Multi-head attention (B=2, L=S=2048, D=1024, H=16) on 8 Trainium2 cores.

Sharding: core c -> batch b = c // 4, head group g = c % 4 (4 heads per core).
W_Q/K/V column-sharded (256 cols per core), W_O row-sharded (256 rows per core);
the 4 partial outputs per batch are summed on the host (plus bias terms).

Per-core pipeline (all big tensors kept transposed so no on-device transposes):
  projections: QT = 0.125*(x Wq + bq)^T, KT = (x Wk + bk)^T (feature-major
    [256, L]); Vaug = [V_h | ones] per head (seq-major, fp16), V bias folded
    out on the host (softmax rows sum to 1 => + bv @ Wo + bo once).
  attention, per (l-tile 512, s-tile 128): S^T = KT^T QT (row-packed pairs of
    heads, K=64, the two 64-row matmuls co-execute in disjoint PE row
    halves); E = exp(S^T) * maskT; T_h += Vaug_h^T E accumulates the head
    output AND its softmax row-sums in one matmul (ones columns act as the
    reducer).

Scheduling: the 64 (lt, st) attention tiles form ONE flat software pipeline
with the AV accumulation running two s-tiles behind the score matmuls -
including across lt boundaries - so the PE never idle-waits on the
ACT exp -> DVE mask chain.  Idle waits re-throttle the PE clock to 1.2 GHz
(HAM activity monitor); in the unpipelined version ~36% of matmuls ran at
half clock.  The QT projection chain for lt+1 is emitted in two halves
mid-lt (PE filler during the ACT-bound steady state).  ACT does exp only;
QT/KT biases are DVE scalar_tensor_tensor; the mask multiply is DVE; the
softmax-sum lane swaps and output DMAs ride the idle Sync DMA queue.
out-projection: out_partial = outT^T Wo_rows, PSUM->SBUF copies split
between ACT (idle after the last exp) and DVE.

All matmul operands fp16 (1 cyc/row); PSUM fp32.  PSUM budget 8 banks =
scores 2x2 + T_h 4x1; projection and output-projection matmuls borrow the
same slots.
"""
from contextlib import ExitStack

import numpy as np

import concourse.bass as bass
import concourse.mybir as mybir
import concourse.tile as tile
from concourse import bacc
from concourse.bass_utils import run_bass_kernel_spmd

F16 = mybir.dt.float16
F32 = mybir.dt.float32

D = 1024          # d_model
H = 16            # heads
DK = 64           # head dim
B, L = 2, 2048
NCORES = 8
HPC = 4           # heads per core
FPC = HPC * DK    # features per core = 256
KD = D // 128     # 8 contraction subtiles for projections
LT, LTW = 4, 512  # l tiles
ST, STW = 16, 128  # s tiles
Exp = mybir.ActivationFunctionType.Exp

_CACHED_NC = None


def _build():
    nc = bacc.Bacc("TRN2", target_bir_lowering=False, debug=False,
                   num_devices=NCORES)
    xT = nc.declare_dram_parameter("xT", [128, KD, L], F16, isOutput=False)
    wq = nc.declare_dram_parameter("wq", [128, KD, FPC], F16, isOutput=False)
    wk = nc.declare_dram_parameter("wk", [128, KD, FPC], F16, isOutput=False)
    wv = nc.declare_dram_parameter("wv", [128, KD, FPC], F16, isOutput=False)
    wo = nc.declare_dram_parameter("wo", [128, 2, D], F16, isOutput=False)
    bq = nc.declare_dram_parameter("bq", [128, 2], F32, isOutput=False)
    bk = nc.declare_dram_parameter("bk", [128, 2], F32, isOutput=False)
    maskT = nc.declare_dram_parameter("maskT", [ST, LT, 128, LTW], F16,
                                      isOutput=False)
    out = nc.declare_dram_parameter("out", [128, ST, D], F16, isOutput=True)

    with tile.TileContext(nc) as tc, ExitStack() as ctx:
        pool = ctx.enter_context(tc.tile_pool(name="pers", bufs=1))
        mpool = ctx.enter_context(tc.tile_pool(name="mpool", bufs=4))
        epool = ctx.enter_context(tc.tile_pool(name="epool", bufs=4))
        rbpool = ctx.enter_context(tc.tile_pool(name="rbpool", bufs=4))
        opool = ctx.enter_context(tc.tile_pool(name="opool", bufs=3))
        scp = ctx.enter_context(tc.tile_pool(name="scp", bufs=2, space="PSUM"))
        tp = ctx.enter_context(tc.tile_pool(name="tp", bufs=1, space="PSUM"))

        xt = pool.tile([128, KD, L], F16)
        wq_sb = pool.tile([128, KD, FPC], F16)
        wk_sb = pool.tile([128, KD, FPC], F16)
        wv_sb = pool.tile([128, KD, FPC], F16)
        wo_sb = pool.tile([128, 2, D], F16)
        bq_sb = pool.tile([128, 2], F32)
        bk_sb = pool.tile([128, 2], F32)
        nc.sync.dma_start(out=wk_sb[:], in_=wk[:])
        nc.sync.dma_start(out=xt[:, 0, :], in_=xT[:, 0, :])
        nc.sync.dma_start(out=wq_sb[:], in_=wq[:])
        nc.sync.dma_start(out=wv_sb[:], in_=wv[:])
        for kd in range(1, KD):
            nc.sync.dma_start(out=xt[:, kd, :], in_=xT[:, kd, :])
        nc.sync.dma_start(out=bk_sb[:], in_=bk[:])
        nc.sync.dma_start(out=bq_sb[:], in_=bq[:])
        nc.sync.dma_start(out=wo_sb[:], in_=wo[:])

        QT = pool.tile([128, 2, L], F16)   # [feat(2x128), l]: Q^T * 0.125
        KT = pool.tile([128, 2, L], F16)
        # Vaug[:, st, h]: even h -> [V_h | 1], odd h -> [1 | V_h]
        Vaug = pool.tile([128, ST, HPC, 128], F16)
        nc.gpsimd.memset(Vaug[:], 1.0)
        outTs = [pool.tile([128, 2, LTW], F16, name=f"outT{i}")
                 for i in range(LT)]

        def emit_kt_chunk(c):
            lsl = slice(c * LTW, (c + 1) * LTW)
            ps = scp.tile([128, 2, LTW], F32, tag="sc", name=f"pk{c}")
            for ft in range(2):
                fsl = slice(ft * 128, (ft + 1) * 128)
                for kd in range(KD):
                    nc.tensor.matmul(ps[:, ft, :], wk_sb[:, kd, fsl],
                                     xt[:, kd, lsl],
                                     start=(kd == 0), stop=(kd == KD - 1))
                nc.vector.scalar_tensor_tensor(
                    KT[:, ft, lsl], ps[:, ft, :], 1.0,
                    bk_sb[:, ft:ft + 1].to_broadcast((128, LTW)),
                    mybir.AluOpType.mult, mybir.AluOpType.add)

        def emit_v_chunk(c):
            for st in range(4 * c, 4 * c + 4):
                ssl = slice(st * STW, (st + 1) * STW)
                psv = tp.tile([128, LTW], F32, tag=f"T{st % 4}", name=f"psv{st}")
                for kd in range(KD):
                    nc.tensor.matmul(psv[:, :FPC], xt[:, kd, ssl],
                                     wv_sb[:, kd, :],
                                     start=(kd == 0), stop=(kd == KD - 1))
                for h in range(HPC):
                    off = 0 if h % 2 == 0 else 64
                    nc.vector.tensor_copy(Vaug[:, st, h, off:off + 64],
                                          psv[:, DK * h:DK * (h + 1)])

        def emit_qt_half(lt, ft):
            lsl = slice(lt * LTW, (lt + 1) * LTW)
            psq = scp.tile([128, 1, LTW], F32, tag="sc", name=f"pq{lt}_{ft}")
            fsl = slice(ft * 128, (ft + 1) * 128)
            for kd in range(KD):
                nc.tensor.matmul(psq[:, 0, :], wq_sb[:, kd, fsl],
                                 xt[:, kd, lsl],
                                 start=(kd == 0), stop=(kd == KD - 1))
            nc.vector.scalar_tensor_tensor(
                QT[:, ft, lsl], psq[:, 0, :], 0.125,
                bq_sb[:, ft:ft + 1].to_broadcast((128, LTW)),
                mybir.AluOpType.mult, mybir.AluOpType.add)

        # ---------------- attention pipeline ----------------
        mks = {}
        Es = {}
        Ts_by_lt = {}

        def issue_mk(lt, st):
            mk = mpool.tile([128, LTW], F16, tag="mk")
            nc.sync.dma_start(out=mk[:], in_=maskT[st, lt])
            mks[(lt, st)] = mk

        def emit_scores(lt, st):
            lsl = slice(lt * LTW, (lt + 1) * LTW)
            ssl = slice(st * STW, (st + 1) * STW)
            mk = mks.pop((lt, st))
            Epair = []
            for pair in range(2):
                sc = scp.tile([128, 2, LTW], F32, tag="sc")
                for i in range(2):
                    nc.tensor.matmul(
                        sc[:, i, :],
                        KT[64 * i:64 * (i + 1), pair, ssl],
                        QT[64 * i:64 * (i + 1), pair, lsl],
                        start=True, stop=True)
                E = epool.tile([128, 2, LTW], F16, tag=f"E{pair}")
                nc.scalar.activation(E[:], sc[:], Exp)
                nc.vector.tensor_mul(
                    E[:], E[:],
                    mk[:, None, :].to_broadcast((128, 2, LTW)))
                Epair.append(E)
            Es[(lt, st)] = Epair

        def emit_av(lt, st):
            Epair = Es.pop((lt, st))
            Ts = Ts_by_lt[lt]
            for pair in range(2):
                for i in range(2):
                    h = 2 * pair + i
                    nc.tensor.matmul(Ts[h][:], Vaug[:, st, h, :],
                                     Epair[pair][:, i, :],
                                     start=(st == 0), stop=(st == ST - 1))

        def emit_norm(lt):
            # reciprocal_approx_fast only works at partition base 0, so
            # route the row sums through lanes 0:64 in both parities.
            Ts = Ts_by_lt.pop(lt)
            for h in range(HPC):
                pair, i = divmod(h, 2)
                av_sl = slice(64 * i, 64 * (i + 1))        # av lanes
                rs_sl = slice(64 * (1 - i), 64 * (2 - i))  # row-sum lanes
                rb = rbpool.tile([128, LTW], F32)
                if i == 0:   # av 0:64, sums 64:128 -> move sums down first
                    nc.vector.tensor_copy(rb[64:128, :], Ts[h][rs_sl, :])
                    nc.sync.dma_start(out=rb[0:64, :], in_=rb[64:128, :])
                    nc.vector.reciprocal_approx_fast(out=rb[0:64, :],
                                                     in_=rb[0:64, :])
                else:        # sums 0:64 -> recip at base 0, then move up
                    nc.vector.reciprocal_approx_fast(out=rb[0:64, :],
                                                     in_=Ts[h][rs_sl, :])
                    nc.sync.dma_start(out=rb[64:128, :], in_=rb[0:64, :])
                nc.vector.tensor_mul(outTs[lt][av_sl, pair, :],
                                     Ts[h][av_sl, :], rb[av_sl, :])

        # prologue: KT chunk 0 + QT(0) first so the exp pipeline starts
        # ~7us in; remaining projection chunks stream under it.
        emit_kt_chunk(0)
        emit_qt_half(0, 0)
        emit_qt_half(0, 1)
        emit_v_chunk(0)
        seq = [(lt, st) for lt in range(LT) for st in range(ST)]
        issue_mk(0, 0)
        issue_mk(0, 1)
        emit_scores(0, 0)
        emit_scores(0, 1)
        for c in range(1, 4):
            emit_kt_chunk(c)
            emit_v_chunk(c)

        for idx, (lt, st) in enumerate(seq):
            if st == 0:
                Ts_by_lt[lt] = [
                    tp.tile([128, LTW], F32, tag=f"T{h}", name=f"T{h}_{lt}")
                    for h in range(HPC)]
            if idx + 2 < len(seq):
                issue_mk(*seq[idx + 2])
                emit_scores(*seq[idx + 2])
            emit_av(lt, st)
            if lt + 1 < LT:
                if st == 8:
                    emit_qt_half(lt + 1, 0)
                elif st == 11:
                    emit_qt_half(lt + 1, 1)
            if st == ST - 1:
                emit_norm(lt)

        # ---------------- output projection ----------------
        for lt8 in range(ST):
            ps3 = scp.tile([128, 2, LTW], F32, tag="sc", name=f"ps3_{lt8}")
            for nf in range(2):
                nsl = slice(nf * 512, (nf + 1) * 512)
                for pair in range(2):
                    nc.tensor.matmul(
                        ps3[:, nf, :],
                        outTs[lt8 // 4][:, pair,
                                        (lt8 % 4) * 128:(lt8 % 4 + 1) * 128],
                        wo_sb[:, pair, nsl],
                        start=(pair == 0), stop=(pair == 1))
            ob = opool.tile([128, D], F16)
            if lt8 % 2 == 0:
                nc.scalar.copy(ob[:], ps3[:])
            else:
                nc.vector.tensor_copy(ob[:], ps3[:])
            nc.sync.dma_start(out=out[:, lt8, :], in_=ob[:])

    nc.compile()
    return nc


def _get_nc():
    global _CACHED_NC
    if _CACHED_NC is None:
        _CACHED_NC = _build()
    return _CACHED_NC


def _prep_core_inputs(c, x, mask, Wq, bq, Wk, bk, Wv, Wo):
    b, g = divmod(c, 4)
    cs = slice(g * FPC, (g + 1) * FPC)

    xT = np.ascontiguousarray(
        x[b].T.reshape(KD, 128, L).transpose(1, 0, 2)).astype(np.float16)
    wq_c = np.ascontiguousarray(
        Wq[:, cs].reshape(KD, 128, FPC).transpose(1, 0, 2)).astype(np.float16)
    wk_c = np.ascontiguousarray(
        Wk[:, cs].reshape(KD, 128, FPC).transpose(1, 0, 2)).astype(np.float16)
    wv_c = np.ascontiguousarray(
        Wv[:, cs].reshape(KD, 128, FPC).transpose(1, 0, 2)).astype(np.float16)
    wo_c = np.ascontiguousarray(
        Wo[cs, :].reshape(2, 128, D).transpose(1, 0, 2)).astype(np.float16)
    bq_c = np.ascontiguousarray(
        (bq[cs] * 0.125).reshape(2, 128).T).astype(np.float32)
    bk_c = np.ascontiguousarray(bk[cs].reshape(2, 128).T).astype(np.float32)
    mT = mask[b].astype(np.float16).T  # [S, L]
    maskT = np.ascontiguousarray(
        mT.reshape(ST, 128, LT, LTW).transpose(0, 2, 1, 3))
    return {"xT": xT, "wq": wq_c, "wk": wk_c, "wv": wv_c, "wo": wo_c,
            "bq": bq_c, "bk": bk_c, "maskT": maskT}


def kernel(x, mask, Wq, bq, Wk, bk, Wv, bv, Wo, bo):
    x = np.asarray(x, np.float32)
    mask = np.asarray(mask)
    Wq, bq = np.asarray(Wq, np.float32), np.asarray(bq, np.float32)
    Wk, bk = np.asarray(Wk, np.float32), np.asarray(bk, np.float32)
    Wv, bv = np.asarray(Wv, np.float32), np.asarray(bv, np.float32)
    Wo, bo = np.asarray(Wo, np.float32), np.asarray(bo, np.float32)

    nc = _get_nc()
    in_maps = [_prep_core_inputs(c, x, mask, Wq, bq, Wk, bk, Wv, Wo)
               for c in range(NCORES)]
    res = run_bass_kernel_spmd(nc, in_maps, list(range(NCORES)))

    const_vec = (bv @ Wo + bo).astype(np.float32)  # A rows sum to 1
    outs = []
    for b in range(B):
        acc = np.zeros((L, D), np.float32)
        for g in range(4):
            part = res.results[4 * b + g]["out"]  # [128, 16, 1024] fp16
            acc += part.transpose(1, 0, 2).reshape(L, D).astype(np.float32)
        acc += const_vec
        outs.append(acc)
    return np.stack(outs)


# revision 22
# speedup vs baseline: 1.2186x; 1.0271x over previous
"""Multi-head attention (B=2, L=S=2048, D=1024, H=16) on 8 Trainium2 cores.

Sharding: core c -> batch b = c // 4, head group g = c % 4 (4 heads per core).
W_Q/K/V column-sharded (256 cols per core), W_O row-sharded (256 rows per core);
the 4 partial outputs per batch are summed on the host (plus bias terms).

Per-core pipeline (all big tensors kept transposed so no on-device transposes):
  projections: QT = 0.125*(x Wq + bq)^T, KT = (x Wk + bk)^T (feature-major
    [256, L]); Vaug = [V_h | ones] per head (seq-major, fp16), V bias folded
    out on the host (softmax rows sum to 1 => + bv @ Wo + bo once).
  attention, per (l-tile 512, s-tile 128): S^T = KT^T QT (row-packed pairs of
    heads, K=64, the two 64-row matmuls co-execute in disjoint PE row
    halves); E = exp(S^T) * maskT; T_h += Vaug_h^T E accumulates the head
    output AND its softmax row-sums in one matmul (ones columns act as the
    reducer).

Scheduling: the 64 (lt, st) attention tiles form ONE flat software pipeline
with the AV accumulation running three s-tiles behind the score matmuls -
including across lt boundaries - so the PE never idle-waits on the
ACT exp -> DVE mask chain.  Idle waits re-throttle the PE clock to 1.2 GHz
(HAM activity monitor); in the unpipelined version ~36% of matmuls ran at
half clock.  The QT projection chain for lt+1 is emitted in two halves
mid-lt and each lt's out-projection is spread through the NEXT lt (PE
filler during the ACT-bound steady state); only lt3's out-projection runs
as a tail.  x arrives via one coarse strided DMA per l-quarter so KT/QT
chunk 0 finish after ~25% of x has landed and the exp pipeline starts
early.  ACT does exp only (+4 tail copies); QT/KT biases are DVE
scalar_tensor_tensor; softmax-sum lane swaps and output stores ride the
Sync DMA queue.

All matmul operands fp16 (1 cyc/row); PSUM fp32.  PSUM budget 8 banks =
scores 2x2 + T_h 4x1; projection and output-projection matmuls borrow the
same slots.
"""
from contextlib import ExitStack

import numpy as np

import concourse.bass as bass
import concourse.mybir as mybir
import concourse.tile as tile
from concourse import bacc
from concourse.bass_utils import run_bass_kernel_spmd

F16 = mybir.dt.float16
F32 = mybir.dt.float32

D = 1024          # d_model
H = 16            # heads
DK = 64           # head dim
B, L = 2, 2048
NCORES = 8
HPC = 4           # heads per core
FPC = HPC * DK    # features per core = 256
KD = D // 128     # 8 contraction subtiles for projections
LT, LTW = 4, 512  # l tiles
ST, STW = 16, 128  # s tiles
Exp = mybir.ActivationFunctionType.Exp

_CACHED_NC = None


def _build():
    nc = bacc.Bacc("TRN2", target_bir_lowering=False, debug=False,
                   num_devices=NCORES)
    xT = nc.declare_dram_parameter("xT", [128, KD, L], F16, isOutput=False)
    wq = nc.declare_dram_parameter("wq", [128, KD, FPC], F16, isOutput=False)
    wk = nc.declare_dram_parameter("wk", [128, KD, FPC], F16, isOutput=False)
    wv = nc.declare_dram_parameter("wv", [128, KD, FPC], F16, isOutput=False)
    wo = nc.declare_dram_parameter("wo", [128, 2, D], F16, isOutput=False)
    bq = nc.declare_dram_parameter("bq", [128, 2], F32, isOutput=False)
    bk = nc.declare_dram_parameter("bk", [128, 2], F32, isOutput=False)
    maskT = nc.declare_dram_parameter("maskT", [ST, LT, 128, LTW], F16,
                                      isOutput=False)
    out = nc.declare_dram_parameter("out", [128, ST, D], F16, isOutput=True)

    with tile.TileContext(nc) as tc, ExitStack() as ctx:
        pool = ctx.enter_context(tc.tile_pool(name="pers", bufs=1))
        mpool = ctx.enter_context(tc.tile_pool(name="mpool", bufs=5))
        epool = ctx.enter_context(tc.tile_pool(name="epool", bufs=5))
        rbpool = ctx.enter_context(tc.tile_pool(name="rbpool", bufs=4))
        opool = ctx.enter_context(tc.tile_pool(name="opool", bufs=3))
        scp = ctx.enter_context(tc.tile_pool(name="scp", bufs=2, space="PSUM"))
        tp = ctx.enter_context(tc.tile_pool(name="tp", bufs=1, space="PSUM"))

        xt = pool.tile([128, KD, L], F16)
        wq_sb = pool.tile([128, KD, FPC], F16)
        wk_sb = pool.tile([128, KD, FPC], F16)
        wv_sb = pool.tile([128, KD, FPC], F16)
        wo_sb = pool.tile([128, 2, D], F16)
        bq_sb = pool.tile([128, 2], F32)
        bk_sb = pool.tile([128, 2], F32)
        # DMA order: l-quarter-major for xT (coarse strided DMAs - each
        # dma_start costs ~600ns of Sync queue time) so the KT-chunk-0 /
        # QT-chunk-0 chains (which contract over all KD but only touch
        # l 0:512) finish after ~25% of x has landed.
        nc.sync.dma_start(out=wk_sb[:], in_=wk[:])
        nc.sync.dma_start(out=xt[:, 0:4, 0:LTW], in_=xT[:, 0:4, 0:LTW])
        nc.sync.dma_start(out=xt[:, 4:KD, 0:LTW], in_=xT[:, 4:KD, 0:LTW])
        nc.sync.dma_start(out=wq_sb[:], in_=wq[:])
        nc.sync.dma_start(out=bk_sb[:], in_=bk[:])
        nc.sync.dma_start(out=bq_sb[:], in_=bq[:])

        QT = pool.tile([128, 2, L], F16)   # [feat(2x128), l]: Q^T * 0.125
        KT = pool.tile([128, 2, L], F16)
        # Vaug[:, st, h]: even h -> [V_h | 1], odd h -> [1 | V_h]
        Vaug = pool.tile([128, ST, HPC, 128], F16)
        nc.gpsimd.memset(Vaug[:], 1.0)
        outTs = [pool.tile([128, 2, LTW], F16, name=f"outT{i}")
                 for i in range(LT)]

        def emit_kt_chunk(c):
            lsl = slice(c * LTW, (c + 1) * LTW)
            ps = scp.tile([128, 2, LTW], F32, tag="sc", name=f"pk{c}")
            for ft in range(2):
                fsl = slice(ft * 128, (ft + 1) * 128)
                for kd in range(KD):
                    nc.tensor.matmul(ps[:, ft, :], wk_sb[:, kd, fsl],
                                     xt[:, kd, lsl],
                                     start=(kd == 0), stop=(kd == KD - 1))
                nc.vector.scalar_tensor_tensor(
                    KT[:, ft, lsl], ps[:, ft, :], 1.0,
                    bk_sb[:, ft:ft + 1].to_broadcast((128, LTW)),
                    mybir.AluOpType.mult, mybir.AluOpType.add)

        def emit_v_chunk(c):
            for st in range(4 * c, 4 * c + 4):
                ssl = slice(st * STW, (st + 1) * STW)
                psv = tp.tile([128, LTW], F32, tag=f"T{st % 4}", name=f"psv{st}")
                for kd in range(KD):
                    nc.tensor.matmul(psv[:, :FPC], xt[:, kd, ssl],
                                     wv_sb[:, kd, :],
                                     start=(kd == 0), stop=(kd == KD - 1))
                for h in range(HPC):
                    off = 0 if h % 2 == 0 else 64
                    nc.vector.tensor_copy(Vaug[:, st, h, off:off + 64],
                                          psv[:, DK * h:DK * (h + 1)])

        def emit_qt_half(lt, ft):
            lsl = slice(lt * LTW, (lt + 1) * LTW)
            psq = scp.tile([128, 1, LTW], F32, tag="sc", name=f"pq{lt}_{ft}")
            fsl = slice(ft * 128, (ft + 1) * 128)
            for kd in range(KD):
                nc.tensor.matmul(psq[:, 0, :], wq_sb[:, kd, fsl],
                                 xt[:, kd, lsl],
                                 start=(kd == 0), stop=(kd == KD - 1))
            nc.vector.scalar_tensor_tensor(
                QT[:, ft, lsl], psq[:, 0, :], 0.125,
                bq_sb[:, ft:ft + 1].to_broadcast((128, LTW)),
                mybir.AluOpType.mult, mybir.AluOpType.add)

        # ---------------- attention pipeline ----------------
        mks = {}
        Es = {}
        Ts_by_lt = {}

        def issue_mk(lt, st):
            mk = mpool.tile([128, LTW], F16, tag="mk")
            nc.sync.dma_start(out=mk[:], in_=maskT[st, lt])
            mks[(lt, st)] = mk

        def emit_scores(lt, st):
            lsl = slice(lt * LTW, (lt + 1) * LTW)
            ssl = slice(st * STW, (st + 1) * STW)
            mk = mks.pop((lt, st))
            Epair = []
            for pair in range(2):
                sc = scp.tile([128, 2, LTW], F32, tag="sc")
                for i in range(2):
                    nc.tensor.matmul(
                        sc[:, i, :],
                        KT[64 * i:64 * (i + 1), pair, ssl],
                        QT[64 * i:64 * (i + 1), pair, lsl],
                        start=True, stop=True)
                E = epool.tile([128, 2, LTW], F16, tag=f"E{pair}")
                nc.scalar.activation(E[:], sc[:], Exp)
                nc.vector.tensor_mul(
                    E[:], E[:],
                    mk[:, None, :].to_broadcast((128, 2, LTW)))
                Epair.append(E)
            Es[(lt, st)] = Epair

        def emit_av(lt, st):
            Epair = Es.pop((lt, st))
            Ts = Ts_by_lt[lt]
            for pair in range(2):
                for i in range(2):
                    h = 2 * pair + i
                    nc.tensor.matmul(Ts[h][:], Vaug[:, st, h, :],
                                     Epair[pair][:, i, :],
                                     start=(st == 0), stop=(st == ST - 1))

        def emit_norm(lt):
            # reciprocal_approx_fast only works at partition base 0, so
            # route the row sums through lanes 0:64 in both parities.
            Ts = Ts_by_lt.pop(lt)
            for h in range(HPC):
                pair, i = divmod(h, 2)
                av_sl = slice(64 * i, 64 * (i + 1))        # av lanes
                rs_sl = slice(64 * (1 - i), 64 * (2 - i))  # row-sum lanes
                rb = rbpool.tile([128, LTW], F32)
                if i == 0:   # av 0:64, sums 64:128 -> move sums down first
                    nc.vector.tensor_copy(rb[64:128, :], Ts[h][rs_sl, :])
                    nc.sync.dma_start(out=rb[0:64, :], in_=rb[64:128, :])
                    nc.vector.reciprocal_approx_fast(out=rb[0:64, :],
                                                     in_=rb[0:64, :])
                else:        # sums 0:64 -> recip at base 0, then move up
                    nc.vector.reciprocal_approx_fast(out=rb[0:64, :],
                                                     in_=Ts[h][rs_sl, :])
                    nc.sync.dma_start(out=rb[64:128, :], in_=rb[0:64, :])
                nc.vector.tensor_mul(outTs[lt][av_sl, pair, :],
                                     Ts[h][av_sl, :], rb[av_sl, :])

        def emit_outproj_chunk(lt, c):
            lt8 = 4 * lt + c
            ps3 = scp.tile([128, 2, LTW], F32, tag="sc", name=f"ps3_{lt8}")
            for nf in range(2):
                nsl = slice(nf * 512, (nf + 1) * 512)
                for pair in range(2):
                    nc.tensor.matmul(
                        ps3[:, nf, :],
                        outTs[lt][:, pair, c * 128:(c + 1) * 128],
                        wo_sb[:, pair, nsl],
                        start=(pair == 0), stop=(pair == 1))
            ob = opool.tile([128, D], F16)
            if lt8 >= 12:  # tail chunks: ACT is idle after the last exp
                nc.scalar.copy(ob[:], ps3[:])
            else:
                nc.vector.tensor_copy(ob[:], ps3[:])
            nc.sync.dma_start(out=out[:, lt8, :], in_=ob[:])

        # prologue: KT chunk 0 + QT(0) first, then the first three score
        # tiles so the exp pipeline starts early; remaining input DMAs and
        # projection chunks stream under it.
        emit_kt_chunk(0)
        emit_qt_half(0, 0)
        emit_qt_half(0, 1)
        seq = [(lt, st) for lt in range(LT) for st in range(ST)]
        issue_mk(0, 0)
        issue_mk(0, 1)
        issue_mk(0, 2)
        emit_scores(0, 0)
        emit_scores(0, 1)
        emit_scores(0, 2)
        nc.sync.dma_start(out=wv_sb[:], in_=wv[:])
        for q in range(1, 4):
            qsl = slice(q * LTW, (q + 1) * LTW)
            nc.sync.dma_start(out=xt[:, :, qsl], in_=xT[:, :, qsl])
        nc.sync.dma_start(out=wo_sb[:], in_=wo[:])
        emit_v_chunk(0)
        for c in range(1, 4):
            emit_kt_chunk(c)
            emit_v_chunk(c)

        for idx, (lt, st) in enumerate(seq):
            if st == 0:
                Ts_by_lt[lt] = [
                    tp.tile([128, LTW], F32, tag=f"T{h}", name=f"T{h}_{lt}")
                    for h in range(HPC)]
            if idx + 3 < len(seq):
                issue_mk(*seq[idx + 3])
                emit_scores(*seq[idx + 3])
            emit_av(lt, st)
            if lt + 1 < LT:
                if st == 8:
                    emit_qt_half(lt + 1, 0)
                elif st == 11:
                    emit_qt_half(lt + 1, 1)
            if lt >= 1 and st in (2, 6, 10, 14):
                emit_outproj_chunk(lt - 1, (st - 2) // 4)
            if st == ST - 1:
                emit_norm(lt)

        for c in range(4):
            emit_outproj_chunk(LT - 1, c)

    nc.compile()
    return nc


def _get_nc():
    global _CACHED_NC
    if _CACHED_NC is None:
        _CACHED_NC = _build()
    return _CACHED_NC


def _prep_core_inputs(c, x, mask, Wq, bq, Wk, bk, Wv, Wo):
    b, g = divmod(c, 4)
    cs = slice(g * FPC, (g + 1) * FPC)

    xT = np.ascontiguousarray(
        x[b].T.reshape(KD, 128, L).transpose(1, 0, 2)).astype(np.float16)
    wq_c = np.ascontiguousarray(
        Wq[:, cs].reshape(KD, 128, FPC).transpose(1, 0, 2)).astype(np.float16)
    wk_c = np.ascontiguousarray(
        Wk[:, cs].reshape(KD, 128, FPC).transpose(1, 0, 2)).astype(np.float16)
    wv_c = np.ascontiguousarray(
        Wv[:, cs].reshape(KD, 128, FPC).transpose(1, 0, 2)).astype(np.float16)
    wo_c = np.ascontiguousarray(
        Wo[cs, :].reshape(2, 128, D).transpose(1, 0, 2)).astype(np.float16)
    bq_c = np.ascontiguousarray(
        (bq[cs] * 0.125).reshape(2, 128).T).astype(np.float32)
    bk_c = np.ascontiguousarray(bk[cs].reshape(2, 128).T).astype(np.float32)
    mT = mask[b].astype(np.float16).T  # [S, L]
    maskT = np.ascontiguousarray(
        mT.reshape(ST, 128, LT, LTW).transpose(0, 2, 1, 3))
    return {"xT": xT, "wq": wq_c, "wk": wk_c, "wv": wv_c, "wo": wo_c,
            "bq": bq_c, "bk": bk_c, "maskT": maskT}


def kernel(x, mask, Wq, bq, Wk, bk, Wv, bv, Wo, bo):
    x = np.asarray(x, np.float32)
    mask = np.asarray(mask)
    Wq, bq = np.asarray(Wq, np.float32), np.asarray(bq, np.float32)
    Wk, bk = np.asarray(Wk, np.float32), np.asarray(bk, np.float32)
    Wv, bv = np.asarray(Wv, np.float32), np.asarray(bv, np.float32)
    Wo, bo = np.asarray(Wo, np.float32), np.asarray(bo, np.float32)

    nc = _get_nc()
    in_maps = [_prep_core_inputs(c, x, mask, Wq, bq, Wk, bk, Wv, Wo)
               for c in range(NCORES)]
    res = run_bass_kernel_spmd(nc, in_maps, list(range(NCORES)))

    const_vec = (bv @ Wo + bo).astype(np.float32)  # A rows sum to 1
    outs = []
    for b in range(B):
        acc = np.zeros((L, D), np.float32)
        for g in range(4):
            part = res.results[4 * b + g]["out"]  # [128, 16, 1024] fp16
            acc += part.transpose(1, 0, 2).reshape(L, D).astype(np.float32)
        acc += const_vec
        outs.append(acc)
    return np.stack(outs)


# revision 26
# speedup vs baseline: 1.2418x; 1.0190x over previous
"""Multi-head attention (B=2, L=S=2048, D=1024, H=16) on 8 Trainium2 cores.

Sharding: core c -> batch b = c // 4, head group g = c % 4 (4 heads per core).
W_Q/K/V column-sharded (256 cols per core), W_O row-sharded (256 rows per core);
the 4 partial outputs per batch are summed on the host (plus bias terms).

Per-core pipeline (all big tensors kept transposed so no on-device transposes):
  projections: QT = 0.125*(x Wq + bq)^T, KT = (x Wk + bk)^T (feature-major
    [256, L]); Vaug = [V_h | ones] per head (seq-major, fp16), V bias folded
    out on the host (softmax rows sum to 1 => + bv @ Wo + bo once).
  attention, per (l-tile 512, s-tile 128): S^T = KT^T QT (row-packed pairs of
    heads, K=64, the two 64-row matmuls co-execute in disjoint PE row
    halves); E = exp(S^T) * maskT; T_h += Vaug_h^T E accumulates the head
    output AND its softmax row-sums in one matmul (ones columns act as the
    reducer).

Scheduling: the 64 (lt, st) attention tiles form ONE flat software pipeline
with the AV accumulation running four s-tiles behind the score matmuls -
including across lt boundaries - so the PE never idle-waits on the
ACT exp -> DVE mask chain.  Idle waits re-throttle the PE clock to 1.2 GHz
(HAM activity monitor); in the unpipelined version ~36% of matmuls ran at
half clock.  The QT projection chain for lt+1 is emitted in two halves
mid-lt and each lt's out-projection is spread through the NEXT lt (PE
filler during the ACT-bound steady state); only lt3's out-projection runs
as a tail.  x arrives via one coarse strided DMA per l-quarter so KT/QT
chunk 0 finish after ~25% of x has landed and the exp pipeline starts
early.  ACT does exp only (+4 tail copies); QT/KT biases are DVE
scalar_tensor_tensor; softmax-sum lane swaps and output stores ride the
Sync DMA queue.

All matmul operands fp16 (1 cyc/row); PSUM fp32.  PSUM budget 8 banks =
scores 2x2 + T_h 4x1; projection and output-projection matmuls borrow the
same slots.
"""
from contextlib import ExitStack

import numpy as np

import concourse.bass as bass
import concourse.mybir as mybir
import concourse.tile as tile
from concourse import bacc
from concourse.bass_utils import run_bass_kernel_spmd

F16 = mybir.dt.float16
F32 = mybir.dt.float32

D = 1024          # d_model
H = 16            # heads
DK = 64           # head dim
B, L = 2, 2048
NCORES = 8
HPC = 4           # heads per core
FPC = HPC * DK    # features per core = 256
KD = D // 128     # 8 contraction subtiles for projections
LT, LTW = 4, 512  # l tiles
ST, STW = 16, 128  # s tiles
Exp = mybir.ActivationFunctionType.Exp

_CACHED_NC = None


def _build():
    nc = bacc.Bacc("TRN2", target_bir_lowering=False, debug=False,
                   num_devices=NCORES)
    xT = nc.declare_dram_parameter("xT", [128, KD, L], F16, isOutput=False)
    wq = nc.declare_dram_parameter("wq", [128, KD, FPC], F16, isOutput=False)
    wk = nc.declare_dram_parameter("wk", [128, KD, FPC], F16, isOutput=False)
    wv = nc.declare_dram_parameter("wv", [128, KD, FPC], F16, isOutput=False)
    wo = nc.declare_dram_parameter("wo", [128, 2, D], F16, isOutput=False)
    bq = nc.declare_dram_parameter("bq", [128, 2], F32, isOutput=False)
    bk = nc.declare_dram_parameter("bk", [128, 2], F32, isOutput=False)
    maskT = nc.declare_dram_parameter("maskT", [ST, LT, 128, LTW], F16,
                                      isOutput=False)
    out = nc.declare_dram_parameter("out", [128, ST, D], F16, isOutput=True)

    with tile.TileContext(nc) as tc, ExitStack() as ctx:
        pool = ctx.enter_context(tc.tile_pool(name="pers", bufs=1))
        mpool = ctx.enter_context(tc.tile_pool(name="mpool", bufs=7))
        epool = ctx.enter_context(tc.tile_pool(name="epool", bufs=7))
        rbpool = ctx.enter_context(tc.tile_pool(name="rbpool", bufs=4))
        opool = ctx.enter_context(tc.tile_pool(name="opool", bufs=3))
        scp = ctx.enter_context(tc.tile_pool(name="scp", bufs=2, space="PSUM"))
        tp = ctx.enter_context(tc.tile_pool(name="tp", bufs=1, space="PSUM"))

        xt = pool.tile([128, KD, L], F16)
        wq_sb = pool.tile([128, KD, FPC], F16)
        wk_sb = pool.tile([128, KD, FPC], F16)
        wv_sb = pool.tile([128, KD, FPC], F16)
        wo_sb = pool.tile([128, 2, D], F16)
        bq_sb = pool.tile([128, 2], F32)
        bk_sb = pool.tile([128, 2], F32)
        # DMA order: l-quarter-major for xT (coarse strided DMAs - each
        # dma_start costs ~600ns of Sync queue time) so the KT-chunk-0 /
        # QT-chunk-0 chains (which contract over all KD but only touch
        # l 0:512) finish after ~25% of x has landed.
        nc.sync.dma_start(out=wk_sb[:], in_=wk[:])
        nc.sync.dma_start(out=xt[:, 0:4, 0:LTW], in_=xT[:, 0:4, 0:LTW])
        nc.sync.dma_start(out=xt[:, 4:KD, 0:LTW], in_=xT[:, 4:KD, 0:LTW])
        nc.sync.dma_start(out=wq_sb[:], in_=wq[:])
        nc.sync.dma_start(out=bk_sb[:], in_=bk[:])
        nc.sync.dma_start(out=bq_sb[:], in_=bq[:])

        QT = pool.tile([128, 2, L], F16)   # [feat(2x128), l]: Q^T * 0.125
        KT = pool.tile([128, 2, L], F16)
        # Vaug[:, st, h]: even h -> [V_h | 1], odd h -> [1 | V_h]
        Vaug = pool.tile([128, ST, HPC, 128], F16)
        nc.gpsimd.memset(Vaug[:], 1.0)
        outTs = [pool.tile([128, 2, LTW], F16, name=f"outT{i}")
                 for i in range(LT)]

        def emit_kt_chunk(c):
            lsl = slice(c * LTW, (c + 1) * LTW)
            ps = scp.tile([128, 2, LTW], F32, tag="sc", name=f"pk{c}")
            for ft in range(2):
                fsl = slice(ft * 128, (ft + 1) * 128)
                for kd in range(KD):
                    nc.tensor.matmul(ps[:, ft, :], wk_sb[:, kd, fsl],
                                     xt[:, kd, lsl],
                                     start=(kd == 0), stop=(kd == KD - 1))
                nc.vector.scalar_tensor_tensor(
                    KT[:, ft, lsl], ps[:, ft, :], 1.0,
                    bk_sb[:, ft:ft + 1].to_broadcast((128, LTW)),
                    mybir.AluOpType.mult, mybir.AluOpType.add)

        def emit_v_chunk(c):
            for st in range(4 * c, 4 * c + 4):
                ssl = slice(st * STW, (st + 1) * STW)
                psv = tp.tile([128, LTW], F32, tag=f"T{st % 4}", name=f"psv{st}")
                for kd in range(KD):
                    nc.tensor.matmul(psv[:, :FPC], xt[:, kd, ssl],
                                     wv_sb[:, kd, :],
                                     start=(kd == 0), stop=(kd == KD - 1))
                for h in range(HPC):
                    off = 0 if h % 2 == 0 else 64
                    nc.vector.tensor_copy(Vaug[:, st, h, off:off + 64],
                                          psv[:, DK * h:DK * (h + 1)])

        def emit_qt_half(lt, ft):
            lsl = slice(lt * LTW, (lt + 1) * LTW)
            psq = scp.tile([128, 1, LTW], F32, tag="sc", name=f"pq{lt}_{ft}")
            fsl = slice(ft * 128, (ft + 1) * 128)
            for kd in range(KD):
                nc.tensor.matmul(psq[:, 0, :], wq_sb[:, kd, fsl],
                                 xt[:, kd, lsl],
                                 start=(kd == 0), stop=(kd == KD - 1))
            nc.vector.scalar_tensor_tensor(
                QT[:, ft, lsl], psq[:, 0, :], 0.125,
                bq_sb[:, ft:ft + 1].to_broadcast((128, LTW)),
                mybir.AluOpType.mult, mybir.AluOpType.add)

        # ---------------- attention pipeline ----------------
        mks = {}
        Es = {}
        Ts_by_lt = {}

        def issue_mk(lt, st):
            mk = mpool.tile([128, LTW], F16, tag="mk")
            nc.sync.dma_start(out=mk[:], in_=maskT[st, lt])
            mks[(lt, st)] = mk

        def emit_scores(lt, st):
            lsl = slice(lt * LTW, (lt + 1) * LTW)
            ssl = slice(st * STW, (st + 1) * STW)
            mk = mks.pop((lt, st))
            Epair = []
            for pair in range(2):
                sc = scp.tile([128, 2, LTW], F32, tag="sc")
                for i in range(2):
                    nc.tensor.matmul(
                        sc[:, i, :],
                        KT[64 * i:64 * (i + 1), pair, ssl],
                        QT[64 * i:64 * (i + 1), pair, lsl],
                        start=True, stop=True)
                E = epool.tile([128, 2, LTW], F16, tag=f"E{pair}")
                nc.scalar.activation(E[:], sc[:], Exp)
                nc.vector.tensor_mul(
                    E[:], E[:],
                    mk[:, None, :].to_broadcast((128, 2, LTW)))
                Epair.append(E)
            Es[(lt, st)] = Epair

        def emit_av(lt, st):
            Epair = Es.pop((lt, st))
            Ts = Ts_by_lt[lt]
            for pair in range(2):
                for i in range(2):
                    h = 2 * pair + i
                    nc.tensor.matmul(Ts[h][:], Vaug[:, st, h, :],
                                     Epair[pair][:, i, :],
                                     start=(st == 0), stop=(st == ST - 1))

        def emit_norm(lt):
            # reciprocal_approx_fast only works at partition base 0, so
            # route the row sums through lanes 0:64 in both parities.
            Ts = Ts_by_lt.pop(lt)
            for h in range(HPC):
                pair, i = divmod(h, 2)
                av_sl = slice(64 * i, 64 * (i + 1))        # av lanes
                rs_sl = slice(64 * (1 - i), 64 * (2 - i))  # row-sum lanes
                rb = rbpool.tile([128, LTW], F32)
                if i == 0:   # av 0:64, sums 64:128 -> move sums down first
                    nc.vector.tensor_copy(rb[64:128, :], Ts[h][rs_sl, :])
                    nc.sync.dma_start(out=rb[0:64, :], in_=rb[64:128, :])
                    nc.vector.reciprocal_approx_fast(out=rb[0:64, :],
                                                     in_=rb[0:64, :])
                else:        # sums 0:64 -> recip at base 0, then move up
                    nc.vector.reciprocal_approx_fast(out=rb[0:64, :],
                                                     in_=Ts[h][rs_sl, :])
                    nc.sync.dma_start(out=rb[64:128, :], in_=rb[0:64, :])
                nc.vector.tensor_mul(outTs[lt][av_sl, pair, :],
                                     Ts[h][av_sl, :], rb[av_sl, :])

        def emit_outproj_chunk(lt, c):
            lt8 = 4 * lt + c
            ps3 = scp.tile([128, 2, LTW], F32, tag="sc", name=f"ps3_{lt8}")
            for nf in range(2):
                nsl = slice(nf * 512, (nf + 1) * 512)
                for pair in range(2):
                    nc.tensor.matmul(
                        ps3[:, nf, :],
                        outTs[lt][:, pair, c * 128:(c + 1) * 128],
                        wo_sb[:, pair, nsl],
                        start=(pair == 0), stop=(pair == 1))
            ob = opool.tile([128, D], F16)
            if lt8 >= 12:  # tail chunks: ACT is idle after the last exp
                nc.scalar.copy(ob[:], ps3[:])
            else:
                nc.vector.tensor_copy(ob[:], ps3[:])
            nc.sync.dma_start(out=out[:, lt8, :], in_=ob[:])

        # prologue: KT chunk 0 + QT(0) first, then the first three score
        # tiles so the exp pipeline starts early; remaining input DMAs and
        # projection chunks stream under it.
        emit_kt_chunk(0)
        emit_qt_half(0, 0)
        emit_qt_half(0, 1)
        seq = [(lt, st) for lt in range(LT) for st in range(ST)]
        issue_mk(0, 0)
        issue_mk(0, 1)
        issue_mk(0, 2)
        issue_mk(0, 3)
        emit_scores(0, 0)
        emit_scores(0, 1)
        emit_scores(0, 2)
        emit_scores(0, 3)
        nc.sync.dma_start(out=wv_sb[:], in_=wv[:])
        for q in range(1, 4):
            qsl = slice(q * LTW, (q + 1) * LTW)
            nc.sync.dma_start(out=xt[:, :, qsl], in_=xT[:, :, qsl])
        nc.sync.dma_start(out=wo_sb[:], in_=wo[:])
        emit_v_chunk(0)
        for c in range(1, 4):
            emit_kt_chunk(c)
            emit_v_chunk(c)

        for idx, (lt, st) in enumerate(seq):
            if st == 0:
                Ts_by_lt[lt] = [
                    tp.tile([128, LTW], F32, tag=f"T{h}", name=f"T{h}_{lt}")
                    for h in range(HPC)]
            if idx + 4 < len(seq):
                issue_mk(*seq[idx + 4])
                emit_scores(*seq[idx + 4])
            emit_av(lt, st)
            if lt + 1 < LT:
                if st == 8:
                    emit_qt_half(lt + 1, 0)
                elif st == 11:
                    emit_qt_half(lt + 1, 1)
            if lt >= 1 and st in (2, 6, 10, 14):
                emit_outproj_chunk(lt - 1, (st - 2) // 4)
            if st == ST - 1:
                emit_norm(lt)

        for c in range(4):
            emit_outproj_chunk(LT - 1, c)

    nc.compile()
    return nc


def _get_nc():
    global _CACHED_NC
    if _CACHED_NC is None:
        _CACHED_NC = _build()
    return _CACHED_NC


def _prep_core_inputs(c, x, mask, Wq, bq, Wk, bk, Wv, Wo):
    b, g = divmod(c, 4)
    cs = slice(g * FPC, (g + 1) * FPC)

    xT = np.ascontiguousarray(
        x[b].T.reshape(KD, 128, L).transpose(1, 0, 2)).astype(np.float16)
    wq_c = np.ascontiguousarray(
        Wq[:, cs].reshape(KD, 128, FPC).transpose(1, 0, 2)).astype(np.float16)
    wk_c = np.ascontiguousarray(
        Wk[:, cs].reshape(KD, 128, FPC).transpose(1, 0, 2)).astype(np.float16)
    wv_c = np.ascontiguousarray(
        Wv[:, cs].reshape(KD, 128, FPC).transpose(1, 0, 2)).astype(np.float16)
    wo_c = np.ascontiguousarray(
        Wo[cs, :].reshape(2, 128, D).transpose(1, 0, 2)).astype(np.float16)
    bq_c = np.ascontiguousarray(
        (bq[cs] * 0.125).reshape(2, 128).T).astype(np.float32)
    bk_c = np.ascontiguousarray(bk[cs].reshape(2, 128).T).astype(np.float32)
    mT = mask[b].astype(np.float16).T  # [S, L]
    maskT = np.ascontiguousarray(
        mT.reshape(ST, 128, LT, LTW).transpose(0, 2, 1, 3))
    return {"xT": xT, "wq": wq_c, "wk": wk_c, "wv": wv_c, "wo": wo_c,
            "bq": bq_c, "bk": bk_c, "maskT": maskT}


def kernel(x, mask, Wq, bq, Wk, bk, Wv, bv, Wo, bo):
    x = np.asarray(x, np.float32)
    mask = np.asarray(mask)
    Wq, bq = np.asarray(Wq, np.float32), np.asarray(bq, np.float32)
    Wk, bk = np.asarray(Wk, np.float32), np.asarray(bk, np.float32)
    Wv, bv = np.asarray(Wv, np.float32), np.asarray(bv, np.float32)
    Wo, bo = np.asarray(Wo, np.float32), np.asarray(bo, np.float32)

    nc = _get_nc()
    in_maps = [_prep_core_inputs(c, x, mask, Wq, bq, Wk, bk, Wv, Wo)
               for c in range(NCORES)]
    res = run_bass_kernel_spmd(nc, in_maps, list(range(NCORES)))

    const_vec = (bv @ Wo + bo).astype(np.float32)  # A rows sum to 1
    outs = []
    for b in range(B):
        acc = np.zeros((L, D), np.float32)
        for g in range(4):
            part = res.results[4 * b + g]["out"]  # [128, 16, 1024] fp16
            acc += part.transpose(1, 0, 2).reshape(L, D).astype(np.float32)
        acc += const_vec
        outs.append(acc)
    return np.stack(outs)
